# revision 1
# baseline (speedup 1.0000x reference)
"""Trainium2 kernel for nn_HardNegativeContrastiveLoss.

Math note (exact, not an approximation): the reference masks only the
(i, B+i)/(B+i, i) positive pairs of the similarity matrix but leaves the
diagonal unmasked.  After row-normalization every diagonal entry is
z_r.z_r / T = 1/T, and every off-diagonal entry is cos(z_r, z_j)/T < 1/T,
so hardest_neg[r] == 1/T for every row.  The relu argument
1/T + margin - pos is then always >= 1/T + margin - 1 > 0, hence

    loss = 1/T + margin - mean_i( z1_i . z2_i / (||z1_i|| ||z2_i||) )

This kernel therefore computes the per-row cosine between z1 and z2 —
a pure memory-bound row reduction over the 8 MB of input, sharded
row-wise across the 8 NeuronCores (1024 rows of z1+z2 per core).  Each
core emits its 128 partial sums of cosines; the host combines them.

Work split per core (raw Bass, manual semaphores):
  SP    : four half loads (HWDGE ring) + result store
  ACT   : warm-up (hides the cold activation-table load), z1 squares,
          final sqrt
  DVE   : per-group z1.z2 dots (fused multiply+reduce), z1 norm
          reduces, final normalize chain
"""

import os
import sys
from contextlib import ExitStack

import numpy as np

for _p in (
    "/root/.axon_site",
    "/root/.axon_site/_ro/trn_rl_repo",
    "/root/.axon_site/_ro/pypackages",
    "/opt/trn_rl_repo",
):
    if os.path.isdir(_p) and _p not in sys.path:
        sys.path.append(_p)

import concourse.bass as bass
import concourse.mybir as mybir
from concourse import bass_utils

B, D = 8192, 128
N_CORES = 8
ROWS = B // N_CORES  # rows of z1 (and of z2) handled per core
S = ROWS // 128      # row-groups of 128 per core
H = S // 2
TEMPERATURE = 0.1
MARGIN = 0.5

_cache = {}


def _build():
    f32 = mybir.dt.float32
    mult = mybir.AluOpType.mult
    X = mybir.AxisListType.X
    nc = bass.Bass()
    z1p = nc.declare_dram_parameter("z1c", [ROWS, D], f32, isOutput=False)
    z2p = nc.declare_dram_parameter("z2c", [ROWS, D], f32, isOutput=False)
    outp = nc.declare_dram_parameter("partial", [128, 1], f32, isOutput=True)

    # Partition p holds rows 8p..8p+7, so the per-partition DMA source is
    # one contiguous 4 KB run of HBM.
    z1_ap = z1p[:].rearrange("(p s) d -> p s d", p=128)  # [128, S, D]
    z2_ap = z2p[:].rearrange("(p s) d -> p s d", p=128)

    with ExitStack() as ctx:
        z1t = ctx.enter_context(nc.sbuf_tensor([128, S * D], f32))
        z2t = ctx.enter_context(nc.sbuf_tensor([128, S * D], f32))
        z1sq = ctx.enter_context(nc.sbuf_tensor([128, S * D], f32))
        vscr = ctx.enter_context(nc.sbuf_tensor([128, S * D], f32))
        gscr = ctx.enter_context(nc.sbuf_tensor([128, S * D], f32))
        dots = ctx.enter_context(nc.sbuf_tensor([128, S], f32))
        n1 = ctx.enter_context(nc.sbuf_tensor([128, S], f32))
        n2 = ctx.enter_context(nc.sbuf_tensor([128, S], f32))
        nsq = ctx.enter_context(nc.sbuf_tensor([128, S], f32))
        nrm = ctx.enter_context(nc.sbuf_tensor([128, S], f32))
        rec = ctx.enter_context(nc.sbuf_tensor([128, S], f32))
        pos = ctx.enter_context(nc.sbuf_tensor([128, S], f32))
        rowsum = ctx.enter_context(nc.sbuf_tensor([128, 1], f32))
        wtile = ctx.enter_context(nc.sbuf_tensor([128, 1], f32))
        z1a_sem = ctx.enter_context(nc.semaphore("z1a_sem"))
        z1b_sem = ctx.enter_context(nc.semaphore("z1b_sem"))
        z2a_sem = ctx.enter_context(nc.semaphore("z2a_sem"))
        z2b_sem = ctx.enter_context(nc.semaphore("z2b_sem"))
        st_sem = ctx.enter_context(nc.semaphore("st_sem"))
        act_sem = ctx.enter_context(nc.semaphore("act_sem"))
        dve_sem = ctx.enter_context(nc.semaphore("dve_sem"))
        done_sem = ctx.enter_context(nc.semaphore("done_sem"))
        block = ctx.enter_context(nc.Block())

        ones = nc.const_aps.scalar_like(1.0, wtile[:, :])

        def dot_group(s):
            return nc.vector.scalar_tensor_tensor(
                out=vscr[:, s * D : (s + 1) * D],
                in0=z1t[:, s * D : (s + 1) * D],
                scalar=1.0,
                in1=z2t[:, s * D : (s + 1) * D],
                op0=mult,
                op1=mult,
                accum_out=dots[:, s : s + 1],
            )

        @block.sync
        def _(sync):
            sync.dma_start(out=z1t[:, : H * D], in_=z1_ap[:, :H, :]).then_inc(
                z1a_sem, 16
            )
            sync.dma_start(out=z2t[:, : H * D], in_=z2_ap[:, :H, :]).then_inc(
                z2a_sem, 16
            )
            sync.dma_start(out=z1t[:, H * D :], in_=z1_ap[:, H:, :]).then_inc(
                z1b_sem, 16
            )
            sync.dma_start(out=z2t[:, H * D :], in_=z2_ap[:, H:, :]).then_inc(
                z2b_sem, 16
            )
            sync.wait_ge(done_sem, 1)
            sync.dma_start(out=outp[:], in_=rowsum[:, :]).then_inc(st_sem, 16)

        @block.scalar
        def _(scalar):
            # a1: warm-up -- pays the cold activation-table load during the
            # DMA window instead of on the critical path
            nc.scalar.square(wtile[:, :], ones).then_inc(act_sem, 1)
            # a2/a3: z1 squares per half
            scalar.wait_ge(z1a_sem, 16)
            nc.scalar.square(z1sq[:, : H * D], z1t[:, : H * D]).then_inc(act_sem, 1)
            scalar.wait_ge(z2a_sem, 16)
            nc.scalar.square(gscr[:, : H * D], z2t[:, : H * D]).then_inc(act_sem, 1)
            scalar.wait_ge(z1b_sem, 16)
            nc.scalar.square(z1sq[:, H * D :], z1t[:, H * D :]).then_inc(act_sem, 1)
            scalar.wait_ge(z2b_sem, 16)
            nc.scalar.square(gscr[:, H * D :], z2t[:, H * D :]).then_inc(act_sem, 1)
            # a6: sqrt(n1*n2) once DVE has produced nsq (dve op 13)
            scalar.wait_ge(dve_sem, 13)
            nc.scalar.sqrt(nrm[:, :], nsq[:, :]).then_inc(act_sem, 1)

        @block.vector
        def _(vector):
            # v1..v4: dots h0 (fused multiply+reduce)
            vector.wait_ge(z1a_sem, 16)
            vector.wait_ge(z2a_sem, 16)
            for s in range(H):
                dot_group(s).then_inc(dve_sem, 1)
            # v5: z1 norm reduce, first half
            vector.wait_ge(act_sem, 2)
            nc.vector.reduce_sum(
                n1[:, :H],
                z1sq[:, : H * D].rearrange("p (s d) -> p s d", d=D),
                axis=X,
            ).then_inc(dve_sem, 1)
            # v6: z2 norm reduce, first half
            vector.wait_ge(act_sem, 3)
            nc.vector.reduce_sum(
                n2[:, :H],
                gscr[:, : H * D].rearrange("p (s d) -> p s d", d=D),
                axis=X,
            ).then_inc(dve_sem, 1)
            # v7..v10: dots h1
            vector.wait_ge(z1b_sem, 16)
            vector.wait_ge(z2b_sem, 16)
            for s in range(H, S):
                dot_group(s).then_inc(dve_sem, 1)
            # v11: z1 norm reduce, second half
            vector.wait_ge(act_sem, 4)
            nc.vector.reduce_sum(
                n1[:, H:],
                z1sq[:, H * D :].rearrange("p (s d) -> p s d", d=D),
                axis=X,
            ).then_inc(dve_sem, 1)
            # v12: z2 norm reduce, second half
            vector.wait_ge(act_sem, 5)
            nc.vector.reduce_sum(
                n2[:, H:],
                gscr[:, H * D :].rearrange("p (s d) -> p s d", d=D),
                axis=X,
            ).then_inc(dve_sem, 1)
            # v13: nsq = n1 * n2
            vector.wait_ge(dve_sem, 12)
            nc.vector.tensor_mul(nsq[:, :], n1[:, :], n2[:, :]).then_inc(dve_sem, 1)
            # v14: rec = 1/sqrt(n1*n2); v15: rowsum = sum_s dots*rec (fused)
            vector.wait_ge(act_sem, 6)
            nc.vector.reciprocal(rec[:, :], nrm[:, :]).then_inc(dve_sem, 1)
            vector.wait_ge(dve_sem, 14)
            nc.vector.scalar_tensor_tensor(
                out=pos[:, :],
                in0=dots[:, :],
                scalar=1.0,
                in1=rec[:, :],
                op0=mult,
                op1=mult,
                accum_out=rowsum[:, :],
            ).then_inc(done_sem, 1)

    return nc


def kernel(z1: np.ndarray, z2: np.ndarray) -> np.ndarray:
    z1 = np.ascontiguousarray(np.asarray(z1, dtype=np.float32))
    z2 = np.ascontiguousarray(np.asarray(z2, dtype=np.float32))
    assert z1.shape == (B, D) and z2.shape == (B, D)

    if "nc" not in _cache:
        _cache["nc"] = _build()
    nc = _cache["nc"]

    core_ids = list(range(N_CORES))
    in_maps = [
        {
            "z1c": z1[c * ROWS : (c + 1) * ROWS],
            "z2c": z2[c * ROWS : (c + 1) * ROWS],
        }
        for c in core_ids
    ]
    res = bass_utils.run_bass_kernel_spmd(nc, in_maps, core_ids)
    total = np.float64(0.0)
    for c in core_ids:
        total += np.sum(res.results[c]["partial"].astype(np.float64))
    loss = 1.0 / TEMPERATURE + MARGIN - total / float(B)
    return np.asarray(loss, dtype=np.float32)



# revision 2
# speedup vs baseline: 1.2693x; 1.2693x over previous
"""Trainium2 kernel for nn_HardNegativeContrastiveLoss (optimized).

Math note (exact, not an approximation): the reference masks only the
(i, B+i)/(B+i, i) positive pairs of the similarity matrix but leaves the
diagonal unmasked.  After row-normalization every diagonal entry is 1/T,
and every off-diagonal entry is < 1/T, so hardest_neg[r] == 1/T for every
row and

    loss = 1/T + margin - mean_i cos(z1_i, z2_i)

Per core (1024 rows of z1 and z2 = 1 MB of HBM): compute per row-group g
(128 rows across 128 partitions) three reductions over D=128 columns:
dot_g = sum z1*z2, n1_g = sum z1^2, n2_g = sum z2^2, using fused
multiply+accumulate ops (scalar_tensor_tensor on DVE, Square+accum_out on
ACT).  The host combines cos = dot/sqrt(n1*n2) and takes the mean.

Schedule (tuned against the TimelineSim cost model):
  - inputs are repacked host-side into one partition-major param
    zc[128, 2048] (partition p holds rows 8p..8p+7 of both tensors,
    groups interleaved z1g0,z2g0,z1g1,...), so DMA chunk boundaries and
    arrival order are free choices;
  - 5 SP HWDGE DMAs sized 512/512/512/384/128 columns keep the shared
    DMA engines saturated end-to-end (transfers are back-to-back) while
    the last chunk is a single 64 KB group, minimizing the work that
    must wait for the final +900ns DMA-completion-semaphore latency;
  - 24 reduction ops are split DVE (8 dots + 9 norms) / ACT (7 norms),
    emitted in arrival order so both engines run dense with no idle
    gaps until the tail;
  - all results accumulate into red[128, 24]; one SP store DMA ships
    them when the done-semaphore hits 24.
"""

import os
import sys
from contextlib import ExitStack

import numpy as np

for _p in (
    "/root/.axon_site",
    "/root/.axon_site/_ro/trn_rl_repo",
    "/root/.axon_site/_ro/pypackages",
    "/opt/trn_rl_repo",
):
    if os.path.isdir(_p) and _p not in sys.path:
        sys.path.append(_p)

import concourse.bass as bass
import concourse.mybir as mybir
from concourse import bass_utils

B, D = 8192, 128
N_CORES = 8
ROWS = B // N_CORES
G = ROWS // 128
NCOL = 2 * G * D
TEMPERATURE = 0.1
MARGIN = 0.5

# column layout: j even -> z1 group j//2, j odd -> z2 group j//2
CONFIG = {
    # (col_lo, col_hi) in 128-col group units; all SP-issued, in this order
    "dmas": [(0, 4), (4, 8), (8, 12), (12, 15), (15, 16)],
    # DVE: all 8 dots + 9 norms, in arrival order (dense, no stalls)
    "dve": [("dot", 0), ("dot", 1), ("n", 0), ("n", 1),
            ("dot", 2), ("dot", 3), ("n", 5), ("n", 6),
            ("dot", 4), ("dot", 5), ("n", 8), ("n", 10),
            ("dot", 6), ("n", 12), ("n", 14),
            ("dot", 7), ("n", 15)],
    # ACT: 7 norms via Square+accum_out (engine 292ns + accum read 187ns)
    "act": [("n", 2), ("n", 3), ("n", 4), ("n", 7),
            ("n", 9), ("n", 11), ("n", 13)],
}

_cache = {}


def _build():
    f32 = mybir.dt.float32
    mult = mybir.AluOpType.mult
    cfg = CONFIG
    dmas = cfg["dmas"]

    cover = {}
    for k, (j0, j1) in enumerate(dmas):
        for j in range(j0, j1):
            cover[j] = k
    assert sorted(cover) == list(range(2 * G))

    def req(task):
        if task[0] == "dot":
            g = task[1]
            return max(cover[2 * g], cover[2 * g + 1])
        return cover[task[1]]

    def red_col(task):
        return task[1] if task[0] == "dot" else 8 + task[1]

    n_accum = len(cfg["dve"]) + len(cfg["act"])
    assert n_accum == 24
    want = sorted([("dot", g) for g in range(G)] + [("n", j) for j in range(2 * G)])
    assert sorted(cfg["dve"] + cfg["act"]) == want

    nc = bass.Bass()
    zp = nc.declare_dram_parameter("zc", [128, NCOL], f32, isOutput=False)
    outp = nc.declare_dram_parameter("red", [128, 24], f32, isOutput=True)

    with ExitStack() as ctx:
        zt = ctx.enter_context(nc.sbuf_tensor([128, NCOL], f32))
        red = ctx.enter_context(nc.sbuf_tensor([128, 24], f32))
        scr_v = ctx.enter_context(nc.sbuf_tensor([128, 128], f32))
        scr_a = ctx.enter_context(nc.sbuf_tensor([128, 128], f32))
        dsem = [ctx.enter_context(nc.semaphore(f"d{k}")) for k in range(len(dmas))]
        done = ctx.enter_context(nc.semaphore("done"))
        st = ctx.enter_context(nc.semaphore("st"))
        block = ctx.enter_context(nc.Block())

        def zin(j):
            return zt[:, j * D : (j + 1) * D]

        def emit_ops(eng_name, tasks, waiter):
            last_req = -1
            for t in tasks:
                k = req(t)
                assert k >= last_req, (eng_name, t)
                if k > last_req:
                    waiter.wait_ge(dsem[k], 16)
                    last_req = k
                c = red_col(t)
                if t[0] == "dot":
                    g = t[1]
                    nc.vector.scalar_tensor_tensor(
                        out=scr_v[:, :], in0=zin(2 * g), scalar=1.0,
                        in1=zin(2 * g + 1), op0=mult, op1=mult,
                        accum_out=red[:, c : c + 1],
                    ).then_inc(done, 1)
                elif eng_name == "act":
                    nc.scalar.activation(
                        scr_a[:, :], zin(t[1]),
                        mybir.ActivationFunctionType.Square,
                        accum_out=red[:, c : c + 1],
                    ).then_inc(done, 1)
                else:
                    nc.vector.scalar_tensor_tensor(
                        out=scr_v[:, :], in0=zin(t[1]), scalar=1.0,
                        in1=zin(t[1]), op0=mult, op1=mult,
                        accum_out=red[:, c : c + 1],
                    ).then_inc(done, 1)

        @block.sync
        def _(sync):
            for k, (j0, j1) in enumerate(dmas):
                sync.dma_start(
                    out=zt[:, j0 * D : j1 * D], in_=zp[:, j0 * D : j1 * D]
                ).then_inc(dsem[k], 16)
            sync.wait_ge(done, n_accum)
            sync.dma_start(out=outp[:], in_=red[:, :]).then_inc(st, 16)

        @block.vector
        def _(v):
            emit_ops("dve", cfg["dve"], v)

        @block.scalar
        def _(s):
            emit_ops("act", cfg["act"], s)

    return nc


def kernel(z1: np.ndarray, z2: np.ndarray) -> np.ndarray:
    z1 = np.ascontiguousarray(np.asarray(z1, dtype=np.float32))
    z2 = np.ascontiguousarray(np.asarray(z2, dtype=np.float32))
    assert z1.shape == (B, D) and z2.shape == (B, D)

    if "nc" not in _cache:
        _cache["nc"] = _build()
    nc = _cache["nc"]

    core_ids = list(range(N_CORES))
    in_maps = []
    for c in core_ids:
        z1c = z1[c * ROWS : (c + 1) * ROWS].reshape(128, G, D)
        z2c = z2[c * ROWS : (c + 1) * ROWS].reshape(128, G, D)
        zc = np.empty((128, 2 * G, D), dtype=np.float32)
        zc[:, 0::2, :] = z1c
        zc[:, 1::2, :] = z2c
        in_maps.append({"zc": zc.reshape(128, NCOL)})

    res = bass_utils.run_bass_kernel_spmd(nc, in_maps, core_ids)

    total = np.float64(0.0)
    for c in core_ids:
        red = res.results[c]["red"].astype(np.float64)  # [128, 24]
        dots = red[:, 0:8]
        n1 = red[:, 8:24][:, 0::2]
        n2 = red[:, 8:24][:, 1::2]
        cos = dots / np.sqrt(n1 * n2)
        total += cos.sum()
    loss = 1.0 / TEMPERATURE + MARGIN - total / float(B)
    return np.asarray(loss, dtype=np.float32)
